# revision 4
# baseline (speedup 1.0000x reference)
"""Bass/Trainium2 kernel for nn_DCDicl (DSBlock forward).

Per sample: Q = Unfold_pad4(x)^T @ Unfold_pad4(x) (+ a*I), P = U^T Yz (+ a*d),
D = cho_solve(Q, P).  The dominant FLOPs (the 25.6 GFLOP/sample Gram matrix)
run on 8 NeuronCores: data-parallel over the 4 samples x 2 halves of the
10000-row contraction.  Host does the unfold layout, the tiny P (64 MFLOP),
and the 1600x1600 solve.
"""

import sys

import numpy as np

if "/opt/trn_rl_repo" not in sys.path:
    sys.path.append("/opt/trn_rl_repo")

N, C_IN, C_OUT, H, W, DS = 4, 64, 4, 96, 96, 5
K = C_IN * DS * DS            # 1600
KP = 1664                     # 13 * 128, padded column count
ROWS = 100 * 100              # unfold output positions
HALF = 5120                   # 40 * 128 rows per core (2 halves of 10000, padded)
KCH = HALF // 128             # 40 k-chunks
NT = 256                      # n-tile width (psum free dim)
N_NT = KP // NT               # 6.5 -> 7 handled below
M_MT = KP // 128              # 13 m-tiles

_CACHED = {}


def _build_nc():
    """Raw-Bass double-buffered Gram kernel.

    All input DMAs increment ONE shared dma semaphore (order-independent
    cumulative count), so every consumer needs at most 2 sync waits —
    the hardware per-instruction wait-command limit that Tile's scheduler
    blew through for this pattern.
    """
    from contextlib import ExitStack

    import concourse.bass as bass
    import concourse.mybir as mybir

    nc = bass.Bass()
    u_dram = nc.dram_tensor("u", [HALF, KP], mybir.dt.float32, kind="ExternalInput")
    q_dram = nc.dram_tensor("q", [KP, KP], mybir.dt.float32, kind="ExternalOutput")

    n_nt = (KP + NT - 1) // NT  # 7; last n-tile is 128 wide
    m_his = [min(2 * (n + 1), M_MT) for n in range(n_nt)]
    # schedule tables: per block b -> (n, m, nt, dma count before PE may run)
    blocks = []
    din = 0
    for n in range(n_nt):
        din += KCH  # rhs strip chunks
        for m in range(m_his[n]):
            din += KCH  # lhs chunks
            blocks.append((n, m, min(NT, KP - n * NT), din))
    nblocks = len(blocks)
    cumb = np.cumsum([0] + m_his)  # blocks completed before strip n

    with ExitStack() as ctx:
        rhs_b = [
            ctx.enter_context(nc.sbuf_tensor(f"rhs{i}", [128, KCH, NT], mybir.dt.float32))
            for i in range(2)
        ]
        lhs_b = [
            ctx.enter_context(nc.sbuf_tensor(f"lhs{i}", [128, KCH, 128], mybir.dt.float32))
            for i in range(2)
        ]
        stage = [
            ctx.enter_context(nc.sbuf_tensor(f"stage{i}", [128, NT], mybir.dt.float32))
            for i in range(2)
        ]
        psum = [
            ctx.enter_context(nc.psum_tensor(f"ps{i}", [128, NT], mybir.dt.float32))
            for i in range(2)
        ]
        dma_sem = ctx.enter_context(nc.semaphore("dma_sem"))
        pe_sem = ctx.enter_context(nc.semaphore("pe_sem"))
        ve_sem = ctx.enter_context(nc.semaphore("ve_sem"))
        gp_sem = ctx.enter_context(nc.semaphore("gp_sem"))
        block = ctx.enter_context(nc.Block())

        @block.sync
        def _(sync):
            b = 0
            for n in range(n_nt):
                nt = min(NT, KP - n * NT)
                if n >= 2:  # rhs buffer reused from strip n-2
                    sync.wait_ge(pe_sem, int(cumb[n - 1]))
                for c in range(KCH):
                    sync.dma_start(
                        out=rhs_b[n % 2][:, c, :nt],
                        in_=u_dram[c * 128:(c + 1) * 128, n * NT:n * NT + nt],
                    ).then_inc(dma_sem, 16)
                for m in range(m_his[n]):
                    if b >= 2:  # lhs buffer reused from block b-2
                        sync.wait_ge(pe_sem, b - 1)
                    for c in range(KCH):
                        sync.dma_start(
                            out=lhs_b[b % 2][:, c, :],
                            in_=u_dram[c * 128:(c + 1) * 128, m * 128:(m + 1) * 128],
                        ).then_inc(dma_sem, 16)
                    b += 1

        @block.tensor
        def _(tensor):
            for b, (n, m, nt, din_b) in enumerate(blocks):
                tensor.wait_ge(dma_sem, 16 * din_b)
                if b >= 2:  # psum reused after copy of block b-2
                    tensor.wait_ge(ve_sem, b - 1)
                for c in range(KCH):
                    ins = nc.tensor.matmul(
                        psum[b % 2][:, :nt],
                        lhs_b[b % 2][:, c, :],
                        rhs_b[n % 2][:, c, :nt],
                        start=(c == 0),
                        stop=(c == KCH - 1),
                    )
                ins.then_inc(pe_sem, 1)

        @block.vector
        def _(vector):
            for b, (n, m, nt, _) in enumerate(blocks):
                vector.wait_ge(pe_sem, b + 1)
                if b >= 2:  # stage buffer reused after out-DMA of b-2
                    vector.wait_ge(gp_sem, 16 * (b - 1))
                nc.vector.tensor_copy(
                    stage[b % 2][:, :nt], psum[b % 2][:, :nt]
                ).then_inc(ve_sem, 1)

        @block.gpsimd
        def _(gpsimd):
            for b, (n, m, nt, _) in enumerate(blocks):
                gpsimd.wait_ge(ve_sem, b + 1)
                gpsimd.dma_start(
                    out=q_dram[m * 128:(m + 1) * 128, n * NT:n * NT + nt],
                    in_=stage[b % 2][:, :nt],
                ).then_inc(gp_sem, 16)

    return nc


def _unfold(x1):
    """x1: [C_in, H, W] -> U [10000, 1600] with U[(g,w'),(i,ph,pw)] = xpad[...]"""
    from numpy.lib.stride_tricks import sliding_window_view

    xp2 = np.pad(x1, ((0, 0), (4, 4), (4, 4)))
    sw = sliding_window_view(xp2, (DS, DS), axis=(1, 2))  # [C,100,100,5,5]
    return np.ascontiguousarray(
        sw.transpose(1, 2, 0, 3, 4).reshape(ROWS, K), dtype=np.float32
    )


def kernel(x, d, y, alpha, reg):
    import time as _time
    _t = {"t0": _time.perf_counter()}

    def _mark(name):
        now = _time.perf_counter()
        print(f"[phase] {name}: {now - _t['t0']:.3f}s", file=sys.stderr)
        _t["t0"] = now

    from concourse import bass_utils

    x = np.asarray(x, dtype=np.float32)
    d = np.asarray(d, dtype=np.float32)
    y = np.asarray(y, dtype=np.float32)
    alpha = np.asarray(alpha, dtype=np.float32)
    reg = np.asarray(reg, dtype=np.float32)

    if "nc" not in _CACHED:
        _CACHED["nc"] = _build_nc()
    nc = _CACHED["nc"]
    _mark("build_nc")

    # Host: build padded unfold matrices and shard over 8 cores.
    in_maps = []
    Us = []
    for s in range(N):
        U = _unfold(x[s, 0])  # [10000, 1600]
        Us.append(U)
        Up = np.zeros((2 * HALF, KP), dtype=np.float32)
        Up[:ROWS, :K] = U
        in_maps.append({"u": np.ascontiguousarray(Up[:HALF])})
        in_maps.append({"u": np.ascontiguousarray(Up[HALF:])})
    _mark("host_unfold")

    res = bass_utils.run_bass_kernel_spmd(nc, in_maps, core_ids=list(range(8)))
    outs = res.results
    _mark("spmd_run")

    a = alpha.reshape(N) * H * W * float(reg[0]) / (DS * DS * C_IN)

    out = np.empty((N, C_OUT, C_IN, DS, DS), dtype=np.float32)
    for s in range(N):
        Qp = outs[2 * s]["q"] + outs[2 * s + 1]["q"]
        Qu = np.triu(Qp[:K, :K].astype(np.float64))
        Q = Qu + np.triu(Qp[:K, :K].astype(np.float64), 1).T
        Q += a[s] * np.eye(K)

        # P = U^T Yz  (+ a * d): Yz is y embedded at offset (2,2) in the 100x100 grid
        Yz = np.zeros((100, 100, C_OUT), dtype=np.float32)
        Yz[2:2 + H, 2:2 + W, :] = y[s, :, 0].transpose(1, 2, 0)
        P = Us[s].T.astype(np.float64) @ Yz.reshape(ROWS, C_OUT).astype(np.float64)
        P += a[s] * d[s].transpose(1, 2, 3, 0).reshape(K, C_OUT)

        D = np.linalg.solve(Q, P)  # SPD, kappa ~ 6
        out[s] = D.reshape(C_IN, DS, DS, C_OUT).transpose(3, 0, 1, 2)
    _mark("host_post")
    return out



# revision 13
# speedup vs baseline: 13.6547x; 13.6547x over previous
"""Bass/Trainium2 kernel for nn_DCDicl (DSBlock forward).

Algorithm: instead of the O(K^2 * R) unfold-Gram (baseline), compute the
all-pairs shift correlation corr[j,i,u,v] = sum_{h,w} x[j,h,w] *
xpad[i,h+u-4,w+v-4] (8x fewer FLOPs — the Gram is a Toeplitz gather of
corr), plus the U^T y rows for P folded into the same matmuls.

Device (8 cores = 4 samples x 2 w-halves, bf16 in / fp32 psum):
  out[m, (u,i,v)] = sum_{h, w in half} XY[m,h,w] * xpad[i, h+u, w+v]
with contraction over h (96 partitions) and PSUM accumulation over w.
Host: sum halves, gather Q via a sliding-window view, fp32 Cholesky solve.
"""

import sys
import time

import numpy as np

if "/opt/trn_rl_repo" not in sys.path:
    sys.path.append("/opt/trn_rl_repo")

N, C_IN, C_OUT, H, W, DS = 4, 64, 4, 96, 96, 5
K = C_IN * DS * DS          # 1600
NU = 2 * DS - 1             # 9 shifts per axis
M = C_IN + C_OUT            # 68 lhs rows (64 x-channels + 4 y-channels)
WH = W // 2                 # 48 w-columns per core (contraction half)
WV = WH + NU - 1            # 56 w-columns of padded image needed per core
HP = H + 2 * (DS - 1)       # 104 padded rows
NBLK = C_IN + M             # 132 56-wide column blocks (64 image + 68 xys)
COLS = NBLK * WV            # 7392 columns of the packed input
NCORES = 8

_CACHED = {}
_TIMING = True


def _mark(t, name):
    if _TIMING:
        now = time.perf_counter()
        print(f"[phase] {name}: {now - t[0]:.3f}s", file=sys.stderr)
        t[0] = now


def _build_nc():
    import concourse.bass as bass
    import concourse.mybir as mybir
    from concourse.tile import TileContext

    nc = bass.Bass()
    inp = nc.dram_tensor("inp", [HP, COLS], mybir.dt.bfloat16, kind="ExternalInput")
    out = nc.dram_tensor("o", [M, NU * C_IN * NU], mybir.dt.float32, kind="ExternalOutput")

    with TileContext(nc) as tc:
        with (
            tc.tile_pool(name="inp_p", bufs=1) as inp_p,
            tc.tile_pool(name="ps_p", bufs=8, space="PSUM") as ps_p,
            tc.tile_pool(name="st_p", bufs=1) as st_p,
        ):
            # One DMA materializes all 9 u-shifted replicas via an
            # overlapping sliding-window source AP: all_t[h, u, b, w] =
            # inp[h+u, b, w].  A single DMA completion sem keeps every
            # matmul at <=1 attached sync wait (the HW limit).
            all_t = inp_p.tile([H, NU, NBLK, WV], mybir.dt.bfloat16)
            src = inp[:, :]
            v = src.ap
            v.clear()
            v.extend([(COLS, H), (COLS, NU), (WV, NBLK), (1, WV)])
            nc.sync.dma_start(out=all_t[:, :, :, :], in_=src)

            stage = st_p.tile([M, NU * C_IN * NU], mybir.dt.float32)
            for u in range(NU):
                for ihalf in range(2):
                    ps = ps_p.tile([M, 32 * NU], mybir.dt.float32)
                    for wl in range(WH):
                        nc.tensor.matmul(
                            ps[:, :],
                            all_t[:, 0, C_IN:C_IN + M, wl],
                            all_t[:, u, ihalf * 32:(ihalf + 1) * 32, wl:wl + NU],
                            start=(wl == 0),
                            stop=(wl == WH - 1),
                        )
                    col = (u * 2 + ihalf) * 32 * NU
                    nc.vector.tensor_copy(stage[:, col:col + 32 * NU], ps[:, :])
            nc.sync.dma_start(out=out[:, :], in_=stage[:, :])

    _split_multiwait_drains(nc)
    return nc


def _split_multiwait_drains(nc):
    """Walrus rejects instructions carrying more than one attached sync wait.

    Tile's kernel-tail drain waits on every outstanding semaphore in one
    instruction; split it into a chain of single-wait drains.
    """
    import copy

    import concourse.mybir as mybir

    for fobj in nc.m.functions:
        for blk in fobj.blocks:
            insts = blk.instructions
            k = 0
            while k < len(insts):
                inst = insts[k]
                si = inst.sync_info
                if (
                    isinstance(inst, mybir.InstDrain)
                    and si is not None
                    and len(si.on_wait) > 1
                ):
                    waits = list(si.on_wait)
                    for j, w in enumerate(waits[:-1]):
                        d = copy.copy(inst)
                        d.name = f"{inst.name}_w{j}"
                        d.sync_info = mybir.SyncInfo(on_wait=[w], on_update=[])
                        nc.register_instruction(d)
                        insts.insert(k, d)
                        k += 1
                    inst.sync_info = mybir.SyncInfo(
                        on_wait=[waits[-1]], on_update=list(si.on_update)
                    )
                k += 1


def _build_runner():
    """Build the bass module once and return a cached jitted SPMD callable.

    Mirrors bass2jax.run_bass_via_pjrt's multi-core path, but the jitted
    shard_map is constructed a single time so later calls skip
    trace/lower/compile entirely.
    """
    import jax
    import concourse.mybir as mybir
    from concourse.bass2jax import (
        _bass_exec_p,
        install_neuronx_cc_hook,
        partition_id_tensor,
    )
    from jax.experimental.shard_map import shard_map
    from jax.sharding import Mesh, PartitionSpec

    nc = _build_nc()
    if not nc.is_finalized():
        nc.finalize()
    install_neuronx_cc_hook()
    assert nc.dbg_addr is None
    partition_name = (
        nc.partition_id_tensor.name if nc.partition_id_tensor is not None else None
    )

    in_names, out_names, out_avals, zero_shapes = [], [], [], []
    for alloc in nc.m.functions[0].allocations:
        if not isinstance(alloc, mybir.MemoryLocationSet):
            continue
        name = alloc.memorylocations[0].name
        if alloc.kind == "ExternalInput":
            if name != partition_name:
                in_names.append(name)
        elif alloc.kind == "ExternalOutput":
            shape = tuple(alloc.tensor_shape)
            dtype = mybir.dt.np(alloc.dtype)
            out_names.append(name)
            out_avals.append(jax.core.ShapedArray(shape, dtype))
            zero_shapes.append((shape, dtype))
    n_params = len(in_names)
    n_outs = len(out_avals)
    all_names = in_names + out_names
    if partition_name is not None:
        all_names = all_names + [partition_name]

    def _body(*args):
        operands = list(args)
        if partition_name is not None:
            operands.append(partition_id_tensor())
        outs = _bass_exec_p.bind(
            *operands,
            out_avals=tuple(out_avals),
            in_names=tuple(all_names),
            out_names=tuple(out_names),
            lowering_input_output_aliases=(),
            sim_require_finite=True,
            sim_require_nnan=True,
            nc=nc,
        )
        return tuple(outs)

    devices = jax.devices()[:NCORES]
    mesh = Mesh(np.asarray(devices), ("core",))
    donate = tuple(range(n_params, n_params + n_outs))
    sharded = jax.jit(
        shard_map(
            _body,
            mesh=mesh,
            in_specs=(PartitionSpec("core"),) * (n_params + n_outs),
            out_specs=(PartitionSpec("core"),) * n_outs,
            check_rep=False,
        ),
        donate_argnums=donate,
        keep_unused=True,
    )

    def run(in_maps):
        t = [time.perf_counter()]
        concat_in = [
            np.concatenate([np.asarray(m[name]) for m in in_maps], axis=0)
            for name in in_names
        ]
        concat_zeros = [
            np.zeros((NCORES * s[0], *s[1:]), dt) for s, dt in zero_shapes
        ]
        _mark(t, "  run.concat")
        out_arrs = sharded(*concat_in, *concat_zeros)
        _mark(t, "  run.dispatch")
        res = [
            np.asarray(out_arrs[i]).reshape(NCORES, *out_avals[i].shape)
            for i in range(n_outs)
        ]
        _mark(t, "  run.fetch")
        return res

    return run


def _unfold(x1):
    """x1: [C_in, H, W] -> U [10000, 1600] (kept for test.py's oracle)."""
    from numpy.lib.stride_tricks import sliding_window_view

    xp2 = np.pad(x1, ((0, 0), (4, 4), (4, 4)))
    sw = sliding_window_view(xp2, (DS, DS), axis=(1, 2))
    return np.ascontiguousarray(
        sw.transpose(1, 2, 0, 3, 4).reshape(100 * 100, K), dtype=np.float32
    )


def _prep_in_maps(x, y):
    import ml_dtypes

    bf16 = ml_dtypes.bfloat16
    in_maps = []
    for s in range(N):
        xs = x[s, 0]
        ys = y[s, :, 0]
        xy = np.concatenate([xs, ys], axis=0)                   # [68, 96, 96]
        xyT = xy.transpose(1, 0, 2)                             # [96, 68, 96]
        xpad = np.zeros((C_IN, HP, HP), np.float32)
        xpad[:, DS - 1:DS - 1 + H, DS - 1:DS - 1 + W] = xs
        xpfT = xpad.transpose(1, 0, 2)                          # [104, 64, 104]
        for half in range(2):
            packed = np.zeros((HP, NBLK, WV), np.float32)
            packed[:, :C_IN, :] = xpfT[:, :, WH * half:WH * half + WV]
            packed[:H, C_IN:, :WH] = xyT[:, :, WH * half:WH * (half + 1)]
            in_maps.append({"inp": packed.reshape(HP, COLS).astype(bf16)})
    return in_maps


def kernel(x, d, y, alpha, reg):
    from numpy.lib.stride_tricks import sliding_window_view
    from scipy.linalg import cho_factor, cho_solve

    t = [time.perf_counter()]
    x = np.asarray(x, dtype=np.float32)
    d = np.asarray(d, dtype=np.float32)
    y = np.asarray(y, dtype=np.float32)
    alpha = np.asarray(alpha, dtype=np.float32)
    reg = np.asarray(reg, dtype=np.float32)

    if "run" not in _CACHED:
        _CACHED["run"] = _build_runner()
    run = _CACHED["run"]
    _mark(t, "build")

    in_maps = _prep_in_maps(x, y)
    _mark(t, "prep")

    res = run(in_maps)[0]                                        # [8, 68, 5184]
    _mark(t, "spmd_run")

    a = alpha.reshape(N) * H * W * float(reg[0]) / (DS * DS * C_IN)
    out = np.empty((N, C_OUT, C_IN, DS, DS), dtype=np.float32)
    for s in range(N):
        o = res[2 * s] + res[2 * s + 1]                          # [68, 5184]
        # columns are (u, ihalf, i_local, v) -> [m, i, u, v]
        oc = np.ascontiguousarray(
            o.reshape(M, NU, 2, 32, NU).transpose(0, 2, 3, 1, 4)
        ).reshape(M, C_IN, NU, NU)
        corr = oc[:C_IN]                                         # [j, i, u, v]
        p2 = oc[C_IN:]                                           # [co, i, u, v]

        # Q[(j,kh,kw),(i,ph,pw)] = corr[j, i, ph-kh+4, pw-kw+4]
        swv = sliding_window_view(corr, (DS, DS), axis=(2, 3))   # [j,i,a,b,ph,pw]
        Q4 = swv[:, :, ::-1, ::-1, :, :].transpose(0, 2, 3, 1, 4, 5)
        Q = np.ascontiguousarray(Q4).reshape(K, K)
        Q.flat[::K + 1] += a[s]

        P = np.ascontiguousarray(
            p2[:, :, DS - 3:DS + 2, DS - 3:DS + 2].transpose(1, 2, 3, 0)
        ).reshape(K, C_OUT)
        P += a[s] * d[s].transpose(1, 2, 3, 0).reshape(K, C_OUT)

        cf = cho_factor(Q, lower=False, check_finite=False)
        D = cho_solve(cf, P, check_finite=False)
        out[s] = D.reshape(C_IN, DS, DS, C_OUT).transpose(3, 0, 1, 2)
    _mark(t, "host_post")
    return out
